# revision 32
# baseline (speedup 1.0000x reference)
"""Trainium2 kernel for nn_BalancedHamiltonLayer.

Math: out = einsum("btd,rde->bte", x, factors)/sqrt(rank) + bias.
The einsum contracts r as a plain sum, so sum_r (x @ F_r) == x @ (sum_r F_r):
one [16384,2048] @ [2048,2048] GEMM instead of eight.

Distribution over 8 NeuronCores (single SPMD program, NO collectives):
tensor-parallel on the output dim. Core c owns output columns
e in [c*256, (c+1)*256):
  - factors are column-sharded: core c loads fh = factors[:, :, c-slice]
    (bf16, 1 MB per rank, alternating the two HWDGE queues) and reduces
    W_c = sum_r fh[r] with a DVE pair tree. W_c [2048d, 256e] stays
    SBUF-resident for the whole GEMM.
  - x is replicated (host ships the full x^T in bf16 to every core) and
    STREAMED: 32 m-chunks of [2048d, 512m] (2.1 MB each) triple-buffered
    through SBUF (the front is bytes-bound: factors + first chunk at the
    ~310 GB/s aggregate DMA rate set the first-matmul time; everything
    after is matmul-roofline-bound).
  - GEMM per core: out^T[e, m] = W_c^T @ x^T, 1024 matmuls of N=512
    (stationary = W d-tile [128d,128e], moving = x^T [128d,512m]) —
    back-to-back at ~223 ns each, the bf16 1-column-per-cycle stream
    rate. The 1/sqrt(8) scale and bias land in the PSUM eviction
    (DVE tensor_scalar; bias is per-partition in the transposed layout).
  - Each core writes out^T [256e, 16384m] bf16; the host transposes back.

The reduce pools are scoped so the x tiles reuse their SBUF addresses:
the resulting write-after-read dependency sequences the x stream behind
the factor loads, which is exactly the right DMA schedule (concurrent
transfers round-robin at packet granularity, so any early x prefetch
would delay the last factor byte and with it the whole GEMM).

No AllGather / barrier: cores are fully independent, so launch skew and
collective latency never gate the PE. Everything is bf16 (tolerance is
2e-2; bf16 operand quantization contributes ~5e-3 worst-case here).
"""

import math

import ml_dtypes
import numpy as np

B, T, DIM, RANK = 4, 4096, 2048, 8
N_CORES = 8
M = B * T                      # 16384 rows total (replicated to all cores)
EC = DIM // N_CORES            # 256 output cols per core
NT = DIM // 128                # 16 contraction tiles
NJ = M // 512                  # 32 m-chunks of 512
SCALE = 1.0 / math.sqrt(RANK)

_CACHE = {}


def _build():
    import concourse.bacc as bacc
    import concourse.mybir as mybir
    import concourse.tile as tile

    f32 = mybir.dt.float32
    bf16 = mybir.dt.bfloat16
    add = mybir.AluOpType.add
    mult = mybir.AluOpType.mult

    nc = bacc.Bacc(
        "TRN2", target_bir_lowering=False, debug=False, num_devices=N_CORES
    )
    # x^T tiles, replicated: d = t*128+p, m = J*512+j
    xh = nc.dram_tensor("xh", [NJ, 128, NT, 512], bf16, kind="ExternalInput").ap()
    # this core's factor slice: d = t*128+p, e_global = EC*core + e
    fh = nc.dram_tensor("fh", [RANK, 128, NT, EC], bf16, kind="ExternalInput").ap()
    # biasc[p, e2] = bias[EC*core + e2*128 + p]
    biasc = nc.dram_tensor("biasc", [128, 2], f32, kind="ExternalInput").ap()
    # transposed output slice: outT[e_local, m]
    outT = nc.dram_tensor("outT", [EC, M], bf16, kind="ExternalOutput").ap()

    with tile.TileContext(nc) as tc:
        with (
            tc.tile_pool(name="const", bufs=1) as const_pool,
            tc.tile_pool(name="wsb", bufs=1) as wpool,
        ):
            scope = nc.named_scope
            bias_sb = const_pool.tile([128, 2], f32)
            nc.gpsimd.dma_start(bias_sb[:], biasc[:])

            # NOTE: do NOT pre-warm the PE with dummy matmuls during the DMA
            # front — not even briefly. Any pre-GEMM PE burst (tested 6 us
            # and 66 us worth) correlates with the chip's P0 power downclock
            # engaging for the WHOLE GEMM: 259 ns/MM instead of 216 ns
            # (+40 us). The ~7 us HAM cold-ramp on the first real matmuls is
            # the price of staying inside the power envelope.

            # Phase 1: W_c = sum_r fh[r]. Eight 1 MB loads split across the
            # two HWDGE rings, DVE tree adds (fire as pairs arrive).
            W = wpool.tile([128, NT, EC], bf16)
            with tc.tile_pool(name="red", bufs=8) as red_pool:
                with scope("reduce"):
                    fr = []
                    for r in range(RANK):
                        t_ = red_pool.tile([128, NT, EC], bf16, tag="fr")
                        eng = nc.sync if r % 2 == 0 else nc.scalar
                        eng.dma_start(t_[:], fh[r])
                        fr.append(t_)
                    # arrival-ordered in-place chain: each add (2.3 us)
                    # fits inside the ~3 us rank-arrival gap, so W trails
                    # the LAST factor byte by one add instead of a 3-add
                    # tree tail (all 8 tiles stay live — a smaller pool
                    # would stall the loads behind the adds)
                    nc.vector.tensor_add(W[:], fr[0][:], fr[1][:])
                    for r in range(2, RANK):
                        nc.vector.tensor_add(W[:], W[:], fr[r][:])

            # Phase 2: stream x^T chunks, GEMM e-tile by e-tile, evict with
            # scale+bias, store out^T. No cross-core dependencies anywhere.
            with (
                tc.tile_pool(name="xa", bufs=3) as xapool,
                tc.tile_pool(name="osb", bufs=2) as opool,
                tc.tile_pool(name="ps", bufs=3, space="PSUM") as ppool,
            ):
                for J in range(NJ):
                    xa = xapool.tile([128, NT, 512], bf16, tag="xa")
                    if J == 0:
                        # t-quarter DMAs: the first matmuls only need the
                        # leading t tiles, so the GEMM starts a quarter
                        # chunk after the factors instead of a full one;
                        # the short mid-group waits hide inside the HAM
                        # cold-ramp
                        for q in range(4):
                            nc.sync.dma_start(
                                xa[:, 4 * q : 4 * q + 4, :],
                                xh[J, :, 4 * q : 4 * q + 4, :])
                    elif J == 1:
                        nc.scalar.dma_start(xa[:, : NT // 2, :],
                                            xh[J, :, : NT // 2, :])
                        nc.scalar.dma_start(xa[:, NT // 2 :, :],
                                            xh[J, :, NT // 2 :, :])
                    else:
                        eng = nc.sync if J % 2 == 0 else nc.scalar
                        eng.dma_start(xa[:], xh[J])
                    with scope(f"gemm{J}"):
                        ps = ppool.tile([128, 2, 512], f32, tag="ps")
                        for e2 in range(2):
                            for t in range(NT):
                                nc.tensor.matmul(
                                    ps[:, e2, :],
                                    W[:, t, e2 * 128 : (e2 + 1) * 128],
                                    xa[:, t, :],
                                    start=(t == 0),
                                    stop=(t == NT - 1),
                                )
                        osb = opool.tile([128, 2, 512], bf16, tag="osb")
                        for e2 in range(2):
                            nc.vector.tensor_scalar(
                                osb[:, e2, :], ps[:, e2, :],
                                SCALE, bias_sb[:, e2 : e2 + 1], mult, add,
                            )
                        for e2 in range(2):
                            # final stores ride the (by then idle) HWDGE
                            # queues — lower completion latency than SWDGE
                            # on the kernel's critical tail
                            eng_o = (
                                (nc.sync if e2 == 0 else nc.scalar)
                                if J == NJ - 1 else nc.gpsimd
                            )
                            eng_o.dma_start(
                                outT[e2 * 128 : (e2 + 1) * 128,
                                     J * 512 : (J + 1) * 512],
                                osb[:, e2, :],
                            )

    nc.compile()
    return nc


def _get_nc():
    if "nc" not in _CACHE:
        _CACHE["nc"] = _build()
    return _CACHE["nc"]


def _shard(x, factors, bias):
    bf = ml_dtypes.bfloat16
    x_flat = np.ascontiguousarray(x, dtype=np.float32).reshape(M, DIM)
    factors = np.ascontiguousarray(factors, dtype=np.float32)
    bias = np.ascontiguousarray(bias, dtype=np.float32)
    # xh: [J, p, t, m_local] with d = t*128+p, m = J*512+m_local (replicated)
    xh = np.ascontiguousarray(
        x_flat.T.reshape(NT, 128, NJ, 512).transpose(2, 1, 0, 3).astype(bf)
    )
    in_maps = []
    for c in range(N_CORES):
        fc = factors[:, :, c * EC : (c + 1) * EC]       # [r, d, e]
        fhc = np.ascontiguousarray(
            fc.reshape(RANK, NT, 128, EC).transpose(0, 2, 1, 3).astype(bf)
        )
        biasc = np.ascontiguousarray(
            bias[c * EC : (c + 1) * EC].reshape(2, 128).T
        )
        in_maps.append({"xh": xh, "fh": fhc, "biasc": biasc})
    return in_maps


def _run(in_maps, trace=False, trace_cores=None):
    from concourse.bass_utils import run_bass_kernel_spmd

    nc = _get_nc()
    return run_bass_kernel_spmd(
        nc, in_maps, list(range(N_CORES)), trace=trace, trace_cores=trace_cores
    )


def _assemble(res):
    out = np.empty((M, DIM), dtype=np.float32)
    for c in range(N_CORES):
        out[:, c * EC : (c + 1) * EC] = res.results[c]["outT"].T.astype(np.float32)
    return out.reshape(B, T, DIM)


def kernel(x, factors, bias):
    res = _run(_shard(x, factors, bias), trace=False)
    return _assemble(res)


# revision 33
# speedup vs baseline: 1.0455x; 1.0455x over previous
"""Trainium2 kernel for nn_BalancedHamiltonLayer.

Math: out = einsum("btd,rde->bte", x, factors)/sqrt(rank) + bias.
The einsum contracts r as a plain sum, so sum_r (x @ F_r) == x @ (sum_r F_r):
one [16384,2048] @ [2048,2048] GEMM instead of eight.

Distribution over 8 NeuronCores (single SPMD program, NO collectives):
tensor-parallel on the output dim. Core c owns output columns
e in [c*256, (c+1)*256):
  - factors are column-sharded: core c loads fh = factors[:, :, c-slice]
    (bf16, 1 MB per rank, alternating the two HWDGE queues) and reduces
    W_c = sum_r fh[r] with a DVE pair tree. W_c [2048d, 256e] stays
    SBUF-resident for the whole GEMM.
  - x is replicated (host ships the full x^T in bf16 to every core) and
    STREAMED: 32 m-chunks of [2048d, 512m] (2.1 MB each) triple-buffered
    through SBUF (the front is bytes-bound: factors + first chunk at the
    ~310 GB/s aggregate DMA rate set the first-matmul time; everything
    after is matmul-roofline-bound).
  - GEMM per core: out^T[e, m] = W_c^T @ x^T, 1024 matmuls of N=512
    (stationary = W d-tile [128d,128e], moving = x^T [128d,512m]) —
    back-to-back at ~223 ns each, the bf16 1-column-per-cycle stream
    rate. The 1/sqrt(8) scale and bias land in the PSUM eviction
    (DVE tensor_scalar; bias is per-partition in the transposed layout).
  - Each core writes out^T [256e, 16384m] bf16; the host transposes back.

The reduce pools are scoped so the x tiles reuse their SBUF addresses:
the resulting write-after-read dependency sequences the x stream behind
the factor loads, which is exactly the right DMA schedule (concurrent
transfers round-robin at packet granularity, so any early x prefetch
would delay the last factor byte and with it the whole GEMM).

No AllGather / barrier: cores are fully independent, so launch skew and
collective latency never gate the PE. Everything is bf16 (tolerance is
2e-2; bf16 operand quantization contributes ~5e-3 worst-case here).
"""

import math

import ml_dtypes
import numpy as np

B, T, DIM, RANK = 4, 4096, 2048, 8
N_CORES = 8
M = B * T                      # 16384 rows total (replicated to all cores)
EC = DIM // N_CORES            # 256 output cols per core
NT = DIM // 128                # 16 contraction tiles
NJ = M // 512                  # 32 m-chunks of 512
SCALE = 1.0 / math.sqrt(RANK)

_CACHE = {}


def _build():
    import concourse.bacc as bacc
    import concourse.mybir as mybir
    import concourse.tile as tile

    f32 = mybir.dt.float32
    bf16 = mybir.dt.bfloat16
    add = mybir.AluOpType.add
    mult = mybir.AluOpType.mult

    nc = bacc.Bacc(
        "TRN2", target_bir_lowering=False, debug=False, num_devices=N_CORES
    )
    # x^T tiles, replicated: d = t*128+p, m = J*512+j
    xh = nc.dram_tensor("xh", [NJ, 128, NT, 512], bf16, kind="ExternalInput").ap()
    # this core's factor slice: d = t*128+p, e_global = EC*core + e
    fh = nc.dram_tensor("fh", [RANK, 128, NT, EC], bf16, kind="ExternalInput").ap()
    # biasc[p, e2] = bias[EC*core + e2*128 + p]
    biasc = nc.dram_tensor("biasc", [128, 2], f32, kind="ExternalInput").ap()
    # transposed output slice: outT[e_local, m]
    outT = nc.dram_tensor("outT", [EC, M], bf16, kind="ExternalOutput").ap()

    with tile.TileContext(nc) as tc:
        with (
            tc.tile_pool(name="const", bufs=1) as const_pool,
            tc.tile_pool(name="wsb", bufs=1) as wpool,
        ):
            scope = nc.named_scope
            bias_sb = const_pool.tile([128, 2], f32)
            nc.gpsimd.dma_start(bias_sb[:], biasc[:])

            # NOTE: do NOT pre-warm the PE with dummy matmuls during the DMA
            # front — not even briefly. Any pre-GEMM PE burst (tested 6 us
            # and 66 us worth) correlates with the chip's P0 power downclock
            # engaging for the WHOLE GEMM: 259 ns/MM instead of 216 ns
            # (+40 us). The ~7 us HAM cold-ramp on the first real matmuls is
            # the price of staying inside the power envelope.

            # Phase 1: W_c = sum_r fh[r]. Eight 1 MB loads split across the
            # two HWDGE rings, DVE tree adds (fire as pairs arrive).
            W = wpool.tile([128, NT, EC], bf16)
            with tc.tile_pool(name="red", bufs=8) as red_pool:
                with scope("reduce"):
                    fr = []
                    for r in range(RANK):
                        t_ = red_pool.tile([128, NT, EC], bf16, tag="fr")
                        eng = nc.sync if r % 2 == 0 else nc.scalar
                        eng.dma_start(t_[:], fh[r])
                        fr.append(t_)
                    # arrival-ordered in-place chain: each add (2.3 us)
                    # fits inside the ~3 us rank-arrival gap, so W trails
                    # the LAST factor byte by one add instead of a 3-add
                    # tree tail (all 8 tiles stay live — a smaller pool
                    # would stall the loads behind the adds)
                    nc.vector.tensor_add(W[:], fr[0][:], fr[1][:])
                    for r in range(2, RANK):
                        nc.vector.tensor_add(W[:], W[:], fr[r][:])

            # Phase 2: stream x^T chunks, GEMM e-tile by e-tile, evict with
            # scale+bias, store out^T. No cross-core dependencies anywhere.
            with (
                tc.tile_pool(name="xa", bufs=3) as xapool,
                tc.tile_pool(name="osb", bufs=2) as opool,
                tc.tile_pool(name="ps", bufs=3, space="PSUM") as ppool,
            ):
                for J in range(NJ):
                    xa = xapool.tile([128, NT, 512], bf16, tag="xa")
                    if J <= 1:
                        # two t-half DMAs: the first matmuls of the chunk
                        # only need t=0..7, so the GEMM starts half a chunk
                        # earlier (measured first-MM 40.4 us vs 44.1 whole /
                        # 44.5 quartered — finer splits just churn semaphore
                        # lanes without arriving earlier)
                        eng = nc.sync if J == 0 else nc.scalar
                        eng.dma_start(xa[:, : NT // 2, :],
                                      xh[J, :, : NT // 2, :])
                        eng.dma_start(xa[:, NT // 2 :, :],
                                      xh[J, :, NT // 2 :, :])
                    else:
                        eng = nc.sync if J % 2 == 0 else nc.scalar
                        eng.dma_start(xa[:], xh[J])
                    with scope(f"gemm{J}"):
                        ps = ppool.tile([128, 2, 512], f32, tag="ps")
                        for e2 in range(2):
                            for t in range(NT):
                                nc.tensor.matmul(
                                    ps[:, e2, :],
                                    W[:, t, e2 * 128 : (e2 + 1) * 128],
                                    xa[:, t, :],
                                    start=(t == 0),
                                    stop=(t == NT - 1),
                                )
                        osb = opool.tile([128, 2, 512], bf16, tag="osb")
                        for e2 in range(2):
                            nc.vector.tensor_scalar(
                                osb[:, e2, :], ps[:, e2, :],
                                SCALE, bias_sb[:, e2 : e2 + 1], mult, add,
                            )
                        for e2 in range(2):
                            # final stores ride the (by then idle) HWDGE
                            # queues — lower completion latency than SWDGE
                            # on the kernel's critical tail
                            eng_o = (
                                (nc.sync if e2 == 0 else nc.scalar)
                                if J == NJ - 1 else nc.gpsimd
                            )
                            eng_o.dma_start(
                                outT[e2 * 128 : (e2 + 1) * 128,
                                     J * 512 : (J + 1) * 512],
                                osb[:, e2, :],
                            )

    nc.compile()
    return nc


def _get_nc():
    if "nc" not in _CACHE:
        _CACHE["nc"] = _build()
    return _CACHE["nc"]


def _shard(x, factors, bias):
    bf = ml_dtypes.bfloat16
    x_flat = np.ascontiguousarray(x, dtype=np.float32).reshape(M, DIM)
    factors = np.ascontiguousarray(factors, dtype=np.float32)
    bias = np.ascontiguousarray(bias, dtype=np.float32)
    # xh: [J, p, t, m_local] with d = t*128+p, m = J*512+m_local (replicated)
    xh = np.ascontiguousarray(
        x_flat.T.reshape(NT, 128, NJ, 512).transpose(2, 1, 0, 3).astype(bf)
    )
    in_maps = []
    for c in range(N_CORES):
        fc = factors[:, :, c * EC : (c + 1) * EC]       # [r, d, e]
        fhc = np.ascontiguousarray(
            fc.reshape(RANK, NT, 128, EC).transpose(0, 2, 1, 3).astype(bf)
        )
        biasc = np.ascontiguousarray(
            bias[c * EC : (c + 1) * EC].reshape(2, 128).T
        )
        in_maps.append({"xh": xh, "fh": fhc, "biasc": biasc})
    return in_maps


def _run(in_maps, trace=False, trace_cores=None):
    from concourse.bass_utils import run_bass_kernel_spmd

    nc = _get_nc()
    return run_bass_kernel_spmd(
        nc, in_maps, list(range(N_CORES)), trace=trace, trace_cores=trace_cores
    )


def _assemble(res):
    out = np.empty((M, DIM), dtype=np.float32)
    for c in range(N_CORES):
        out[:, c * EC : (c + 1) * EC] = res.results[c]["outT"].T.astype(np.float32)
    return out.reshape(B, T, DIM)


def kernel(x, factors, bias):
    res = _run(_shard(x, factors, bias), trace=False)
    return _assemble(res)
